# revision 2
# baseline (speedup 1.0000x reference)
"""Distributed embedding-lookup kernel (doc2vec PV-DM forward) for 8 trn2 cores.

Math (per batch element b):
    inputs[b,:]  = paragraph_matrix[doc_ids[b]] + mean_c word_matrix[context_ids[b,c]]
    result[b,s]  = dot(inputs[b,:], outputs[:, sample_ids[b,s]])

Sharding: data-parallel over batch (2048 rows/core); paragraph/word/output
tables replicated to every core.  `outputs` is transposed host-side to
[N_WORDS, D] so sampled columns become contiguous 512B row gathers.

Per-core device work per 128-row tile: 19 indirect-DMA row gathers (1 doc +
8 ctx + 10 sample) + DVE reduce/dot.  The per-core time is bounded by SWDGE
descriptor generation on the Pool engine (~0.93us per indirect DMA, measured),
so the kernel keeps the Pool queue saturated with back-to-back gathers and
overlaps all DVE compute and output DMA under it.
"""

import sys

if "/opt/trn_rl_repo" not in sys.path:
    sys.path.insert(0, "/opt/trn_rl_repo")

import numpy as np

N_CORES = 8
B, C, S = 16384, 8, 10
D = 128
P = 128
N_DOCS, N_WORDS = 200000, 100000
BS = B // N_CORES  # 2048 batch rows per core
T = BS // P        # 16 tiles of 128 rows per core

_COMPILED = {}
LAST_RESULT = None


def build_program(reps=1):
    """Build the per-core program; `reps` repeats the whole body inside a
    hardware loop (used by the timing harness; kernel() uses reps=1)."""
    import concourse.bass as bass
    import concourse.tile as tile
    from concourse import bacc, mybir
    from contextlib import ExitStack

    f32 = mybir.dt.float32
    i32 = mybir.dt.int32

    nc = bacc.Bacc(
        "TRN2",
        target_bir_lowering=False,
        debug=False,
        enable_asserts=False,
        num_devices=N_CORES,
    )

    par_d = nc.dram_tensor("par", [N_DOCS, D], f32, kind="ExternalInput").ap()
    wrd_d = nc.dram_tensor("wrd", [N_WORDS, D], f32, kind="ExternalInput").ap()
    outT_d = nc.dram_tensor("outT", [N_WORDS, D], f32, kind="ExternalInput").ap()
    idx_doc_d = nc.dram_tensor("idx_doc", [P, T], i32, kind="ExternalInput").ap()
    idx_ctx_d = nc.dram_tensor("idx_ctx", [P, T * C], i32, kind="ExternalInput").ap()
    idx_smp_d = nc.dram_tensor("idx_smp", [P, T * S], i32, kind="ExternalInput").ap()
    res_d = nc.dram_tensor("res", [P, T * S], f32, kind="ExternalOutput").ap()

    def body(nc, tc, ctx, idx_doc, idx_ctx, idx_smp, gat, cmp_p, red):
        for t in range(T):
            # one gather per row-block; separate tiles so the SWDGE queue
            # never stalls on WAW within a tile
            par = gat.tile([P, D], f32, tag="par", name="par")
            nc.gpsimd.indirect_dma_start(
                out=par[:],
                out_offset=None,
                in_=par_d,
                in_offset=bass.IndirectOffsetOnAxis(
                    ap=idx_doc[:, t : t + 1], axis=0
                ),
            )
            ctxts = []
            for c in range(C):
                ct = gat.tile([P, D], f32, tag=f"ctx{c}", name=f"ctx{c}")
                nc.gpsimd.indirect_dma_start(
                    out=ct[:],
                    out_offset=None,
                    in_=wrd_d,
                    in_offset=bass.IndirectOffsetOnAxis(
                        ap=idx_ctx[:, t * C + c : t * C + c + 1], axis=0
                    ),
                )
                ctxts.append(ct)
            smpts = []
            for s in range(S):
                st = gat.tile([P, D], f32, tag=f"smp{s}", name=f"smp{s}")
                nc.gpsimd.indirect_dma_start(
                    out=st[:],
                    out_offset=None,
                    in_=outT_d,
                    in_offset=bass.IndirectOffsetOnAxis(
                        ap=idx_smp[:, t * S + s : t * S + s + 1], axis=0
                    ),
                )
                smpts.append(st)

            # ctx sum tree: 4 + 2 + 1 adds
            h = []
            for i in range(4):
                hh = cmp_p.tile([P, D], f32, tag=f"h{i}", name=f"h{i}")
                nc.vector.tensor_add(
                    out=hh[:], in0=ctxts[2 * i][:], in1=ctxts[2 * i + 1][:]
                )
                h.append(hh)
            q0 = cmp_p.tile([P, D], f32, tag="q0", name="q0")
            nc.vector.tensor_add(out=q0[:], in0=h[0][:], in1=h[1][:])
            q1 = cmp_p.tile([P, D], f32, tag="q1", name="q1")
            nc.vector.tensor_add(out=q1[:], in0=h[2][:], in1=h[3][:])
            acc = cmp_p.tile([P, D], f32, tag="acc", name="acc")
            nc.vector.tensor_add(out=acc[:], in0=q0[:], in1=q1[:])

            # inp = acc/C + par
            inp = cmp_p.tile([P, D], f32, tag="inp", name="inp")
            nc.vector.scalar_tensor_tensor(
                out=inp[:],
                in0=acc[:],
                scalar=1.0 / C,
                in1=par[:],
                op0=mybir.AluOpType.mult,
                op1=mybir.AluOpType.add,
            )
            # red[p, t*S+s] = sum_d smp_s[p,d] * inp[p,d]
            prod = cmp_p.tile([P, S * D], f32, tag="prod", name="prod")
            for s in range(S):
                nc.vector.scalar_tensor_tensor(
                    out=prod[:, s * D : (s + 1) * D],
                    in0=smpts[s][:],
                    scalar=1.0,
                    in1=inp[:],
                    op0=mybir.AluOpType.mult,
                    op1=mybir.AluOpType.mult,
                    accum_out=red[:, t * S + s : t * S + s + 1],
                )

    with tile.TileContext(nc) as tc, ExitStack() as ctx:
        idxp = ctx.enter_context(tc.tile_pool(name="idxp", bufs=1))
        gat = ctx.enter_context(tc.tile_pool(name="gat", bufs=6))
        cmp_p = ctx.enter_context(tc.tile_pool(name="cmp", bufs=4))
        outp = ctx.enter_context(tc.tile_pool(name="outp", bufs=1))

        idx_doc = idxp.tile([P, T], i32, name="idx_doc")
        nc.sync.dma_start(out=idx_doc[:], in_=idx_doc_d)
        idx_ctx = idxp.tile([P, T * C], i32, name="idx_ctx")
        nc.sync.dma_start(out=idx_ctx[:], in_=idx_ctx_d)
        idx_smp = idxp.tile([P, T * S], i32, name="idx_smp")
        nc.sync.dma_start(out=idx_smp[:], in_=idx_smp_d)

        red = outp.tile([P, T * S], f32, name="red")

        if reps == 1:
            body(nc, tc, ctx, idx_doc, idx_ctx, idx_smp, gat, cmp_p, red)
        else:
            with tc.For_i(0, reps) as _i:
                body(nc, tc, ctx, idx_doc, idx_ctx, idx_smp, gat, cmp_p, red)

        nc.sync.dma_start(out=res_d, in_=red[:])

    nc.compile()
    return nc


def _get_program():
    if "nc" not in _COMPILED:
        _COMPILED["nc"] = build_program(1)
    return _COMPILED["nc"]


def _tile_major(idx: np.ndarray) -> np.ndarray:
    """[BS, k] int -> [P, T*k] int32 where out[p, t*k+c] = idx[t*P+p, c]."""
    idx = np.asarray(idx)
    if idx.ndim == 1:
        idx = idx[:, None]
    k = idx.shape[1]
    return np.ascontiguousarray(
        idx.reshape(T, P, k).transpose(1, 0, 2).reshape(P, T * k).astype(np.int32)
    )


def make_in_maps(doc_ids, context_ids, sample_ids, paragraph_matrix, word_matrix, outputs):
    par = np.ascontiguousarray(np.asarray(paragraph_matrix, dtype=np.float32))
    wrd = np.ascontiguousarray(np.asarray(word_matrix, dtype=np.float32))
    outT = np.ascontiguousarray(np.asarray(outputs, dtype=np.float32).T)
    doc_ids = np.asarray(doc_ids)
    context_ids = np.asarray(context_ids)
    sample_ids = np.asarray(sample_ids)

    in_maps = []
    for k in range(N_CORES):
        sl = slice(k * BS, (k + 1) * BS)
        in_maps.append(
            {
                "par": par,
                "wrd": wrd,
                "outT": outT,
                "idx_doc": _tile_major(doc_ids[sl]),
                "idx_ctx": _tile_major(context_ids[sl]),
                "idx_smp": _tile_major(sample_ids[sl]),
            }
        )
    return in_maps


def unshard_result(res_list):
    """res per core is [P, T*S] with red[p, t*S+s] = result row t*P+p."""
    outs = []
    for k in range(N_CORES):
        r = res_list[k].reshape(P, T, S).transpose(1, 0, 2).reshape(BS, S)
        outs.append(r)
    return np.concatenate(outs, axis=0).astype(np.float32)


def kernel(
    doc_ids,
    context_ids,
    sample_ids,
    paragraph_matrix,
    word_matrix,
    outputs,
) -> np.ndarray:
    global LAST_RESULT
    from concourse.bass_utils import run_bass_kernel_spmd

    nc = _get_program()
    in_maps = make_in_maps(
        doc_ids, context_ids, sample_ids, paragraph_matrix, word_matrix, outputs
    )
    LAST_RESULT = run_bass_kernel_spmd(nc, in_maps, list(range(N_CORES)))
    return unshard_result(
        [LAST_RESULT.results[k]["res"] for k in range(N_CORES)]
    )
